# revision 27
# baseline (speedup 1.0000x reference)
"""Fused attention block (nn_Attention_27865747817251) on 8 trn2 NeuronCores.

Reference math (per batch b):
  y = x @ w_qkv + b_qkv                      # (L, 3D), D=2048, L=2048
  raw reshape (L, 3D) -> (3, NH, L, HD)      # NH=16, HD=128, NO transpose
  => per-head Q/K/V are CONTIGUOUS ranges of y.flatten():
     q_h = flat[(0*NH+h)*L*HD : ...], k_h = flat[(NH+h)*L*HD : ...], ...
  A = softmax((K_h @ Q_h^T)/sqrt(HD), axis=-1);  out_h = A @ V_h
  out_bld[b, i, h*HD:(h+1)*HD] = out_h[i, :];  final = out_bld @ w_out + b_out

Two SPMD launches on 8 cores (the scramble between them runs on host):

Launch A - QKV projection, core k = (batch k//4, column-group k%4):
  y^T chunks [12, 128, L] = (w_qkv col-slice)^T @ x_b^T, in fp8e4
  DoubleRow (256-deep reduction tiles) with a hi+lo residual split at
  matched scales:
      X1=Q(x), X2=Q(x-X1), W1=Q(32w), W2=Q(32w-W1)
      G = X1@W1 + X1@W2 + X2@W1  (one PSUM group; ~bf16 accuracy)
      y' = 16*G = 512*y  (compensated downstream: exp scale /512^2 and
      w_out/512).  0.75x the PE cycles of bf16 at 4x DoubleRow rate.

Launch B - attention + out-proj, core k = (batch, head-group of 4):
  S^T formulation; softmax denominators OFF the PE: exp writes a
  [128, 16, 512] super-tile, an incremental DVE bf16 add-chain sums the
  16 j-tiles as exps land, gpsimd partition_all_reduce sums the
  partitions (replaces the baseline's ones-matmul: -131k PE cycles).
  Out-proj (row-parallel partial) interleaved one i-block behind
  attention so the PE stays fed while Act runs exp.  Host sums the 4
  partials per batch and adds b_out.
"""

from contextlib import ExitStack

import numpy as np
import ml_dtypes

from concourse import bacc
import concourse.mybir as mybir
import concourse.tile as tile
from concourse.bass_utils import run_bass_kernel_spmd
from concourse.bass_isa import ReduceOp
from concourse.alu_op_type import AluOpType

B, L, D = 2, 2048, 2048
NH, HD = 16, 128
HPC = 4                         # heads per core (launch B)
CPC = 12                        # y^T chunks per core (launch A)
KT = D // 128                   # 16 contraction k-tiles
SCALE = 1.0 / float(np.sqrt(HD))
YS = 512.0                      # y' = YS * y leaves launch A
CEXP = SCALE / (YS * YS)        # exp scale on raw score PSUM

F8 = mybir.dt.float8e4
BF = mybir.dt.bfloat16
F32 = mybir.dt.float32
NP_F8 = ml_dtypes.float8_e4m3fn
NP_BF = ml_dtypes.bfloat16
DR = mybir.MatmulPerfMode.DoubleRow

_CACHE = {}


def _build_a():
    """Core k=(b, cg): y'^T chunks [CPC, 128, L] in bf16, y' = 512*y."""
    nc = bacc.Bacc()
    x1 = nc.dram_tensor("x1", [128, KT, L], F8, kind="ExternalInput")
    x2 = nc.dram_tensor("x2", [128, KT, L], F8, kind="ExternalInput")
    w1 = nc.dram_tensor("w1", [128, CPC, KT, 128], F8, kind="ExternalInput")
    w2 = nc.dram_tensor("w2", [128, CPC, KT, 128], F8, kind="ExternalInput")
    yt = nc.dram_tensor("yt", [CPC, 128, L], BF, kind="ExternalOutput")

    with tile.TileContext(nc) as tc, ExitStack() as ctx:
        wp = ctx.enter_context(tc.tile_pool(name="wp", bufs=1))
        xp = ctx.enter_context(tc.tile_pool(name="xp", bufs=2))
        outs = ctx.enter_context(tc.tile_pool(name="outs", bufs=6))
        psg = ctx.enter_context(tc.tile_pool(name="psg", bufs=4, space="PSUM"))

        w1_sb = wp.tile([128, CPC, KT, 128], F8, tag="w1")
        w2_sb = wp.tile([128, CPC, KT, 128], F8, tag="w2")

        # PE warmup during the DMA lead so real matmuls start at full clock
        wu = wp.tile([128, 512], BF, tag="wu")
        nc.vector.memset(wu[:], 0.0)
        for _ in range(34):
            pwu = psg.tile([128, 512], F32, tag="g")
            nc.tensor.matmul(pwu[:], wu[:, 0:128], wu[:], start=True,
                             stop=True)

        # strip-ordered loads; w on the SP queue (before any output DMA is
        # emitted there), x strips on the Act queue -- separate queues so
        # waiting output DMAs never block input issue.  The first chunk's
        # deps (w1 chunk 0, x1 strip 0) come first; chunk0's term order
        # (X1W1, X1W2, X2W1) matches the arrival order of w2c0 and x2s0.
        nc.sync.dma_start(w1_sb[:, 0], w1[:, 0])
        x1s0 = xp.tile([128, KT, 512], F8, tag="x1s", name="x1s0")
        x2s0 = xp.tile([128, KT, 512], F8, tag="x2s", name="x2s0")
        nc.scalar.dma_start(x1s0[:], x1[:, :, 0:512])
        nc.sync.dma_start(w2_sb[:, 0], w2[:, 0])
        nc.scalar.dma_start(x2s0[:], x2[:, :, 0:512])
        nc.sync.dma_start(w1_sb[:, 1], w1[:, 1])
        nc.sync.dma_start(w2_sb[:, 1], w2[:, 1])
        for c in range(2, CPC, 2):
            nc.sync.dma_start(w1_sb[:, c:c + 2], w1[:, c:c + 2])
            nc.sync.dma_start(w2_sb[:, c:c + 2], w2[:, c:c + 2])

        for rb in range(4):
            s0, s1 = rb * 512, (rb + 1) * 512
            if rb == 0:
                x1s, x2s = x1s0, x2s0
            else:
                x1s = xp.tile([128, KT, 512], F8, tag="x1s", name=f"x1s{rb}")
                x2s = xp.tile([128, KT, 512], F8, tag="x2s", name=f"x2s{rb}")
                nc.scalar.dma_start(x1s[:], x1[:, :, s0:s1])
                nc.scalar.dma_start(x2s[:], x2[:, :, s0:s1])

            for c in range(CPC):
                g = psg.tile([128, 512], F32, tag="g")
                for term, (wsb, xsb) in enumerate(
                        ((w1_sb, x1s), (w1_sb, x2s), (w2_sb, x1s))):
                    for kp in range(8):
                        nc.tensor.matmul(
                            g[:],
                            wsb[:, c, 2 * kp:2 * kp + 2, :],
                            xsb[:, 2 * kp:2 * kp + 2, :],
                            start=(term == 0 and kp == 0),
                            stop=(term == 2 and kp == 7),
                            perf_mode=DR,
                        )
                ot = outs.tile([128, 512], BF, tag="o")
                nc.scalar.mul(ot[:], g[:], 16.0)
                nc.sync.dma_start(yt[c, :, s0:s1], ot[:])
    nc.compile()
    return nc


def _build_b():
    """Core (b, g): attention for 4 heads + row-parallel out-proj partial."""
    nc = bacc.Bacc()
    qt = nc.dram_tensor("qt", [128, HPC, L], BF, kind="ExternalInput")
    kt = nc.dram_tensor("kt", [128, HPC, L], BF, kind="ExternalInput")
    v = nc.dram_tensor("v", [128, HPC, L // 128, HD], BF, kind="ExternalInput")
    wo = nc.dram_tensor("wo", [128, HPC, D], BF, kind="ExternalInput")
    fp = nc.dram_tensor("fp", [L, D], F32, kind="ExternalOutput")

    with tile.TileContext(nc) as tc, ExitStack() as ctx:
        singles = ctx.enter_context(tc.tile_pool(name="singles", bufs=1))
        pts = ctx.enter_context(tc.tile_pool(name="pts", bufs=2))
        nrm = ctx.enter_context(tc.tile_pool(name="nrm", bufs=3))
        fout = ctx.enter_context(tc.tile_pool(name="fout", bufs=6))
        psx = ctx.enter_context(tc.tile_pool(name="psx", bufs=2, space="PSUM"))
        pso = ctx.enter_context(tc.tile_pool(name="pso", bufs=2, space="PSUM"))
        psf = ctx.enter_context(tc.tile_pool(name="psf", bufs=2, space="PSUM"))

        qt_sb = singles.tile([128, HPC, L], BF, tag="qt")
        kt_sb = singles.tile([128, HPC, L], BF, tag="kt")
        v_sb = singles.tile([128, HPC, L // 128, HD], BF, tag="v")
        wo_sb = singles.tile([128, HPC, D], BF, tag="wo")
        outT = singles.tile([128, HPC, L], BF, tag="outT")

        # PE warmup during the DMA lead so real matmuls start at full clock
        wu = singles.tile([128, 512], BF, tag="wu")
        nc.vector.memset(wu[:], 0.0)
        for _ in range(14):
            pwu = psx.tile([128, 2, 512], F32, tag="x")
            nc.tensor.matmul(pwu[:, 0, :], wu[:, 0:128], wu[:], start=True,
                             stop=True)

        # per-head loads so head 0's attention starts early; kt sliced so
        # the first score matmul only waits for qt[h0] + kt[h0] slice 0
        for hh in range(HPC):
            if hh == 0:
                for qs in range(4):
                    nc.sync.dma_start(qt_sb[:, 0, qs * 512:(qs + 1) * 512],
                                      qt[:, 0, qs * 512:(qs + 1) * 512])
                    nc.sync.dma_start(kt_sb[:, 0, qs * 512:(qs + 1) * 512],
                                      kt[:, 0, qs * 512:(qs + 1) * 512])
            else:
                nc.sync.dma_start(qt_sb[:, hh, :], qt[:, hh, :])
                for sl in range(4):
                    nc.sync.dma_start(kt_sb[:, hh, sl * 512:(sl + 1) * 512],
                                      kt[:, hh, sl * 512:(sl + 1) * 512])
            nc.sync.dma_start(v_sb[:, hh, :, :], v[:, hh, :, :])
        nc.sync.dma_start(wo_sb[:], wo[:])

        def outproj_piece(ib, rb, tail=False):
            r0 = (ib * 4 + rb) * 128
            for cb in range(4):
                if tail and cb % 2 == 0:
                    pfx = psx.tile([128, 2, 512], F32, tag="x")
                    pf = pfx[:, 0, :]
                elif tail and cb == 1:
                    pf = pso.tile([128, 512], F32, tag="o")
                else:
                    pf = psf.tile([128, 512], F32, tag="f")
                c0 = cb * 512
                for hh in range(HPC):
                    nc.tensor.matmul(
                        pf[:],
                        outT[:, hh, r0:r0 + 128],
                        wo_sb[:, hh, c0:c0 + 512],
                        start=(hh == 0), stop=(hh == HPC - 1),
                    )
                fo = fout.tile([128, 512], F32, tag="fo")
                if cb % 2 == 1:
                    nc.scalar.copy(fo[:], pf[:])
                else:
                    nc.vector.tensor_copy(fo[:], pf[:])
                nc.sync.dma_start(fp[r0:r0 + 128, c0:c0 + 512], fo[:])

        def av_norm(ptile, ps_o, sums, hh, i0, i1):
            for jb in range(16):
                nc.tensor.matmul(
                    ps_o[:],
                    v_sb[:, hh, jb, :],
                    ptile[:, jb, :],
                    start=(jb == 0), stop=(jb == 15),
                )
            sumf = nrm.tile([128, 512], F32, tag="sf")
            nc.gpsimd.partition_all_reduce(sumf[:], sums[:], 128, ReduceOp.add)
            recip = nrm.tile([128, 512], F32, tag="r")
            nc.vector.reciprocal(recip[:], sumf[:])
            nc.vector.tensor_mul(
                out=outT[:, hh, i0:i1], in0=ps_o[:], in1=recip[:])

        pending = None
        for ib in range(4):
            i0, i1 = ib * 512, (ib + 1) * 512
            for hh in range(HPC):
                ptile = pts.tile([128, 16, 512], BF, tag="pt")
                ps_o = pso.tile([128, 512], F32, tag="o")
                sums = nrm.tile([128, 512], BF, tag="s")
                with nc.allow_low_precision(
                        reason="softmax denom j-tile partial sums in bf16; "
                        "fp32 partition allreduce follows"):
                    for jb2 in range(8):
                        ps_x = psx.tile([128, 2, 512], F32, tag="x")
                        for t in range(2):
                            jb = 2 * jb2 + t
                            nc.tensor.matmul(
                                ps_x[:, t, :],
                                qt_sb[:, hh, jb * 128:(jb + 1) * 128],
                                kt_sb[:, hh, i0:i1],
                                start=True, stop=True,
                            )
                        nc.scalar.activation(
                            ptile[:, 2 * jb2:2 * jb2 + 2, :], ps_x[:],
                            mybir.ActivationFunctionType.Exp, scale=CEXP)
                        # j-denominator accumulation rides the exp pipeline
                        if jb2 == 0:
                            nc.vector.tensor_add(
                                sums[:], ptile[:, 0, :], ptile[:, 1, :])
                        else:
                            for t in range(2):
                                nc.vector.tensor_add(
                                    sums[:], sums[:],
                                    ptile[:, 2 * jb2 + t, :])
                if pending is not None:
                    av_norm(*pending)
                if ib > 0:
                    outproj_piece(ib - 1, hh)
                pending = (ptile, ps_o, sums, hh, i0, i1)
        av_norm(*pending)
        for rb in range(4):
            outproj_piece(3, rb, tail=True)
    nc.compile()
    return nc


def _get(name):
    if name not in _CACHE:
        _CACHE[name] = _build_a() if name == "a" else _build_b()
    return _CACHE[name]


def _q8(a):
    return a.astype(NP_F8)


def _prep_a(x, w_qkv):
    """Per-core launch-A inputs; core k = (b, cg)."""
    ins = []
    xq = {}
    for b in range(B):
        xt = np.ascontiguousarray(
            x[b].T.reshape(KT, 128, L).transpose(1, 0, 2))
        x1 = _q8(xt)
        x2 = _q8(xt - x1.astype(np.float32))
        xq[b] = (x1, x2)
    for k in range(8):
        b, cg = k // 4, k % 4
        wsl = w_qkv[:, cg * CPC * 128:(cg + 1) * CPC * 128] * 32.0
        wt = np.ascontiguousarray(
            wsl.reshape(KT, 128, CPC * 128).transpose(1, 0, 2))
        wt = np.ascontiguousarray(
            wt.reshape(128, KT, CPC, 128).transpose(0, 2, 1, 3))
        w1 = _q8(wt)
        w2 = _q8(wt - w1.astype(np.float32))
        ins.append({"x1": xq[b][0], "x2": xq[b][1], "w1": w1, "w2": w2})
    return ins


def _prep_b(ya_list, b_qkv, w_out):
    """ya_list: 8 arrays [CPC, 128, L] (y' = 512*y); per-core B inputs."""
    sec = L * HD
    ins = []
    for b in range(B):
        yb = np.concatenate([ya_list[b * 4 + cg] for cg in range(4)], axis=0)
        if b_qkv.any():
            yb = (yb.astype(np.float32)
                  + YS * b_qkv.reshape(48, 128)[:, :, None]).astype(NP_BF)
        flat = np.ascontiguousarray(yb.transpose(2, 0, 1)).reshape(-1)
        for g in range(4):
            qts, kts, vs = [], [], []
            for hh in range(HPC):
                h = HPC * g + hh
                qh = flat[h * sec:(h + 1) * sec].reshape(L, HD)
                kh = flat[(NH + h) * sec:(NH + h + 1) * sec].reshape(L, HD)
                vh = flat[(2 * NH + h) * sec:(2 * NH + h + 1) * sec].reshape(
                    L, HD)
                qts.append(qh.T)
                kts.append(kh.T)
                vs.append(vh.reshape(L // 128, 128, HD).transpose(1, 0, 2))
            wsl = w_out[512 * g:512 * (g + 1), :]
            wo_h = np.ascontiguousarray(
                wsl.reshape(HPC, 128, D).transpose(1, 0, 2) / YS).astype(NP_BF)
            ins.append({
                "qt": np.ascontiguousarray(np.stack(qts, axis=1)),
                "kt": np.ascontiguousarray(np.stack(kts, axis=1)),
                "v": np.ascontiguousarray(np.stack(vs, axis=1)),
                "wo": wo_h,
            })
    return ins


def kernel(x, w_qkv, b_qkv, w_out, b_out, _timing=None):
    x = np.asarray(x, dtype=np.float32)
    w_qkv = np.asarray(w_qkv, dtype=np.float32)
    b_qkv = np.asarray(b_qkv, dtype=np.float32)
    w_out = np.asarray(w_out, dtype=np.float32)
    b_out = np.asarray(b_out, dtype=np.float32)
    cores = list(range(8))

    res_a = run_bass_kernel_spmd(_get("a"), _prep_a(x, w_qkv), cores)
    ya = [np.asarray(res_a.results[k]["yt"]) for k in range(8)]

    res_b = run_bass_kernel_spmd(_get("b"), _prep_b(ya, b_qkv, w_out), cores)

    out = np.empty((B, L, D), dtype=np.float32)
    for b in range(B):
        acc = np.zeros((L, D), dtype=np.float32)
        for g in range(4):
            acc += np.asarray(res_b.results[b * 4 + g]["fp"])
        out[b] = acc + b_out[None, :]
    return out
